# revision 7
# baseline (speedup 1.0000x reference)
"""Causal self-attention with LoRA (q,v) — Trainium2 Bass kernel, 8 cores.

Sharding: data-parallel over batch (B=2), tensor-parallel over heads
(16 heads -> 4 per core).  Core c handles batch c//4, heads 4*(c%4)..+4.
Each core computes its 256-dim q/k/v projection slice from the full
hidden states and its heads' full 2048x2048 causal attention locally.
No collectives; host does the (layout-only) scatter/gather.

All matmuls run as float32r (TF32-like, full PE rate at N>=256).
"""

import numpy as np

B, T, DM, H = 2, 2048, 1024, 16
HD = 64          # head dim
R = 8            # LoRA rank
NCORES = 8
GPB = 4          # head-groups (cores) per batch
HPC = 4          # heads per core
DPC = HPC * HD   # 256 output dims per core
LORA_SCALE = 2.0  # lora_alpha / r
SM_SCALE = HD ** -0.5  # 0.125

KC = DM // 128   # 8 contraction chunks
MC = DPC // 128  # 2 output-dim chunks (head pairs)
NB = T // 512    # 4 t-blocks for q/k projections
TCH = T // 128   # 16 t-chunks (key chunks)
IBN = T // 256   # 8 query i-blocks

_CACHE = {}


def _build_program():
    from contextlib import ExitStack

    import concourse.bass as bass
    import concourse.tile as tile
    from concourse import bacc, mybir

    f32 = mybir.dt.float32
    f32r = mybir.dt.float32r
    EXP = mybir.ActivationFunctionType.Exp
    COPY = mybir.ActivationFunctionType.Copy

    def r(ap):  # operands reaching matmuls are f32r-dtyped already
        return ap

    nc = bacc.Bacc(
        "TRN2",
        target_bir_lowering=False,
        debug=False,
        enable_asserts=True,
        num_devices=NCORES,
    )

    xT = nc.dram_tensor("xT", [DM, T], f32, kind="ExternalInput").ap()
    wqT = nc.dram_tensor("wqT", [DM, DPC], f32, kind="ExternalInput").ap()
    wkT = nc.dram_tensor("wkT", [DM, DPC], f32, kind="ExternalInput").ap()
    wvT = nc.dram_tensor("wvT", [DM, DPC], f32, kind="ExternalInput").ap()
    aq = nc.dram_tensor("aq", [R, DM], f32, kind="ExternalInput").ap()
    av = nc.dram_tensor("av", [R, DM], f32, kind="ExternalInput").ap()
    bqT = nc.dram_tensor("bqT", [R, DPC], f32, kind="ExternalInput").ap()
    bvT = nc.dram_tensor("bvT", [R, DPC], f32, kind="ExternalInput").ap()
    biasq = nc.dram_tensor("biasq", [DPC], f32, kind="ExternalInput").ap()
    biask = nc.dram_tensor("biask", [DPC], f32, kind="ExternalInput").ap()
    biasv = nc.dram_tensor("biasv", [DPC], f32, kind="ExternalInput").ap()
    amask = nc.dram_tensor("amask", [T], f32, kind="ExternalInput").ap()
    outT = nc.dram_tensor("outT", [DPC, T], f32, kind="ExternalOutput").ap()

    with tile.TileContext(nc) as tc, ExitStack() as ctx:
        const = ctx.enter_context(tc.tile_pool(name="const", bufs=1))
        xpool = ctx.enter_context(tc.tile_pool(name="x", bufs=1))
        wpool = ctx.enter_context(tc.tile_pool(name="w", bufs=1))
        wtmp = ctx.enter_context(tc.tile_pool(name="wtmp", bufs=2))
        qkpool = ctx.enter_context(tc.tile_pool(name="qk", bufs=1))
        vpool = ctx.enter_context(tc.tile_pool(name="v", bufs=1))
        ppool = ctx.enter_context(tc.tile_pool(name="pT", bufs=3))
        opool = ctx.enter_context(tc.tile_pool(name="osb", bufs=3))
        psum = ctx.enter_context(tc.tile_pool(name="psum", bufs=2, space="PSUM"))
        popool = ctx.enter_context(tc.tile_pool(name="po", bufs=2, space="PSUM"))

        # ---------------- constants ----------------
        # Causal staircase (multiplicative, applied after exp):
        # stair[p, m] = 1.0 if m >= p + 128 else 0.0 ; shape [128, 384].
        # For a partial chunk (jb = 2*ib + d, d in {0,1}) the [128, 256]
        # mask slice is stair[:, 128 - 128*d : 384 - 128*d].
        stair = const.tile([128, 384], f32, tag="stair")
        nc.gpsimd.memset(stair[:], 1.0)
        nc.gpsimd.affine_select(
            out=stair[:],
            in_=stair[:],
            compare_op=mybir.AluOpType.is_ge,
            fill=0.0,
            base=-128,
            pattern=[[1, 384]],
            channel_multiplier=-1,
        )

        ones_f = const.tile([1, 128], f32, tag="ones_f")
        nc.vector.memset(ones_f[:], 1.0)
        ones_1xP = const.tile([1, 128], f32r, tag="ones")
        nc.vector.tensor_copy(ones_1xP[:], ones_f[:])

        # em[p, jb] = exp(amask[128*jb + p])
        em_raw = const.tile([128, TCH], f32, tag="em_raw")
        nc.sync.dma_start(em_raw[:], amask.rearrange("(c p) -> p c", p=128))
        em = const.tile([128, TCH], f32, tag="em")
        nc.scalar.activation(em[:], em_raw[:], EXP)

        # biases as [128, 1] per output-dim chunk
        bias_q = []
        bias_k = []
        for mc in range(MC):
            tq = const.tile([128, 1], f32, tag=f"bq{mc}")
            nc.sync.dma_start(tq[:], biasq[bass.ts(mc, 128)].unsqueeze(1))
            bias_q.append(tq)
            tk = const.tile([128, 1], f32, tag=f"bk{mc}")
            nc.sync.dma_start(tk[:], biask[bass.ts(mc, 128)].unsqueeze(1))
            bias_k.append(tk)
        bv_row = const.tile([1, DPC], f32r, tag="bvrow")
        nc.gpsimd.dma_start(bv_row[:], biasv.unsqueeze(0))

        # LoRA operands
        aq_sb = const.tile([R, DM], f32r, tag="aq")
        nc.gpsimd.dma_start(aq_sb[:], aq)
        av_sb = const.tile([R, DM], f32r, tag="av")
        nc.gpsimd.dma_start(av_sb[:], av)
        bqT_sb = const.tile([R, DPC], f32, tag="bqT")
        nc.sync.dma_start(bqT_sb[:], bqT)
        bqT2 = const.tile([R, DPC], f32r, tag="bqT2")
        nc.scalar.activation(bqT2[:], bqT_sb[:], COPY, scale=LORA_SCALE)
        bvT_sb = const.tile([R, DPC], f32, tag="bvT")
        nc.sync.dma_start(bvT_sb[:], bvT)
        bvT2 = const.tile([R, DPC], f32r, tag="bvT2")
        nc.scalar.activation(bvT2[:], bvT_sb[:], COPY, scale=LORA_SCALE)

        # ---------------- load x.T ----------------
        x_sb = []
        for kc in range(KC):
            t = xpool.tile([128, T], f32r, tag=f"x{kc}")
            nc.gpsimd.dma_start(t[:], xT[bass.ts(kc, 128), :])
            x_sb.append(t)

        # ---------------- weights (+ LoRA fold for q, v) ----------------
        def load_folded(w_dram, a_sb, bT2_sb, name):
            """W'.T chunks = W.T + A.T @ (2 B.T), as 8 [128, DPC] tiles."""
            out_tiles = []
            for kc in range(KC):
                raw = wtmp.tile([128, DPC], f32, tag="wtmp")
                nc.sync.dma_start(raw[:], w_dram[bass.ts(kc, 128), :])
                dps = psum.tile([128, DPC], f32, tag="sc")
                nc.tensor.matmul(
                    dps[:],
                    r(a_sb[:, bass.ts(kc, 128)]),
                    r(bT2_sb[:]),
                    start=True,
                    stop=True,
                )
                wt = wpool.tile([128, DPC], f32r, tag=f"{name}{kc}")
                nc.vector.tensor_add(wt[:], raw[:], dps[:])
                out_tiles.append(wt)
            return out_tiles

        wq_sb = load_folded(wqT, aq_sb, bqT2, "wq")
        wv_sb = load_folded(wvT, av_sb, bvT2, "wv")
        wk_sb = []
        for kc in range(KC):
            t = wpool.tile([128, DPC], f32r, tag=f"wk{kc}")
            nc.gpsimd.dma_start(t[:], wkT[bass.ts(kc, 128), :])
            wk_sb.append(t)

        # ---------------- projections ----------------
        # qT/kT: [d, t] with d on partitions; tile mc holds head pair
        # (2mc, 2mc+1): partitions 0-63 = head 2mc, 64-127 = head 2mc+1.
        qT_sb = [qkpool.tile([128, T], f32r, tag=f"qT{mc}", name=f"qT{mc}") for mc in range(MC)]
        kT_sb = [qkpool.tile([128, T], f32r, tag=f"kT{mc}", name=f"kT{mc}") for mc in range(MC)]

        def project_qk(w_tiles, dst, bias, mc):
            for nb in range(NB):
                ps = psum.tile([128, 512], f32, tag="sc")
                for kc in range(KC):
                    nc.tensor.matmul(
                        ps[:],
                        r(w_tiles[kc][:, bass.ts(mc, 128)]),
                        r(x_sb[kc][:, bass.ts(nb, 512)]),
                        start=(kc == 0),
                        stop=(kc == KC - 1),
                    )
                nc.vector.tensor_add(
                    dst[:, bass.ts(nb, 512)],
                    ps[:],
                    bias[:].to_broadcast((128, 512)),
                )

        # v in natural [t, d] orientation, with em scaling and the
        # denominator (em) column appended per head: [128, 4*65].
        v2_sb = [vpool.tile([128, HPC * (HD + 1)], f32r, tag=f"v2{j}", name=f"v2{j}") for j in range(TCH)]

        def project_v():
            for jb in range(TCH):
                ps = psum.tile([128, DPC], f32, tag="sc")
                for kc in range(KC):
                    nc.tensor.matmul(
                        ps[:],
                        r(x_sb[kc][:, bass.ts(jb, 128)]),
                        r(wv_sb[kc][:]),
                        start=(kc == 0),
                        stop=False,
                    )
                # + ones(t) x bias_v  (rank-1 accumulate)
                nc.tensor.matmul(
                    ps[:],
                    r(ones_1xP[:]),
                    r(bv_row[:]),
                    start=False,
                    stop=True,
                )
                v2 = v2_sb[jb]
                em_col = em[:, jb : jb + 1]
                for hl in range(HPC):
                    nc.vector.tensor_mul(
                        v2[:, hl * (HD + 1) : hl * (HD + 1) + HD],
                        ps[:, bass.ts(hl, HD)],
                        em_col.to_broadcast((128, HD)),
                    )
                # em column (denominator weights)
                nc.vector.tensor_copy(
                    v2[:, HD : HPC * (HD + 1) : HD + 1],
                    em_col.to_broadcast((128, HPC)),
                )

        # ---------------- attention for one head pair ----------------
        def attention_pair(pr):
            h0, h1 = 2 * pr, 2 * pr + 1
            qT, kT = qT_sb[pr], kT_sb[pr]
            for ib in range(IBN):
                nch = 2 * ib + 2  # causal key chunks per head
                # split per-head chunks into groups of <=3 (avoid size-1)
                if nch % 3 == 1:
                    sizes = [3] * (nch // 3 - 1) + [2, 2]
                else:
                    sizes = [3] * (nch // 3) + ([nch % 3] if nch % 3 else [])
                po = popool.tile([65, 512], f32, tag="po")
                jb0 = 0
                for gi, gsz in enumerate(sizes):
                    jbs = list(range(jb0, jb0 + gsz))
                    jb0 += gsz
                    width = 2 * gsz * 256
                    ps = psum.tile([128, width], f32, tag="sc")
                    # segment s: heads h0 -> s, h1 -> s + gsz (bank-disjoint
                    # concurrent row-packed pairs)
                    for s, jb in enumerate(jbs):
                        for hl, seg in ((0, s), (1, s + gsz)):
                            nc.tensor.matmul(
                                ps[:, bass.ts(seg, 256)],
                                r(kT[bass.ts(hl, 64), bass.ts(jb, 128)]),
                                r(qT[bass.ts(hl, 64), bass.ts(ib, 256)]),
                                start=True,
                                stop=True,
                            )
                    pT = ppool.tile([128, width], f32r, tag="pT")
                    nc.scalar.activation(pT[:], ps[:], EXP, scale=SM_SCALE)
                    # causal staircase on partial chunks (jb in {2ib, 2ib+1})
                    for s, jb in enumerate(jbs):
                        d = jb - 2 * ib
                        if d >= 0:  # partial diagonal chunk
                            msk = stair[:, 128 - 128 * d : 384 - 128 * d]
                            for seg in (s, s + gsz):
                                nc.vector.tensor_mul(
                                    pT[:, bass.ts(seg, 256)],
                                    pT[:, bass.ts(seg, 256)],
                                    msk,
                                )
                    # PV: outT[d, i] accumulation, denominator col rides along.
                    # All PV matmuls into this po tile form ONE psum
                    # accumulation group (start on first, stop on last);
                    # per-element has_written handles the two column halves.
                    for s, jb in enumerate(jbs):
                        for hl, seg in ((0, s), (1, s + gsz)):
                            nc.tensor.matmul(
                                po[:, bass.ts(hl, 256)],
                                r(v2_sb[jb][:, (2 * pr + hl) * (HD + 1) : (2 * pr + hl + 1) * (HD + 1)]),
                                r(pT[:, bass.ts(seg, 256)]),
                                start=(jb == 0 and hl == 0),
                                stop=(jb == nch - 1 and hl == 1),
                            )
                # normalize: out[:64] / denom (row 64), per column
                rc = opool.tile([1, 512], f32, tag="rc")
                nc.vector.reciprocal(rc[:], po[64:65, :])
                rb = opool.tile([64, 512], f32, tag="rb")
                nc.gpsimd.partition_broadcast(rb[:], rc[:])
                oT = opool.tile([64, 512], f32, tag="oT")
                nc.vector.tensor_mul(oT[:], po[0:64, :], rb[:])
                for hl, h in ((0, h0), (1, h1)):
                    nc.sync.dma_start(
                        outT[h * HD : (h + 1) * HD, bass.ts(ib, 256)],
                        oT[:, bass.ts(hl, 256)],
                    )

        # emission order chosen for overlap: pair-0 q/k first, then v,
        # then pair-0 attention (its exp tail overlaps pair-1 projections).
        project_qk(wq_sb, qT_sb[0], bias_q[0], 0)
        project_qk(wk_sb, kT_sb[0], bias_k[0], 0)
        project_v()
        attention_pair(0)
        project_qk(wq_sb, qT_sb[1], bias_q[1], 1)
        project_qk(wk_sb, kT_sb[1], bias_k[1], 1)
        attention_pair(1)

    nc.compile()
    return nc


def _shard_inputs(inputs):
    """Full inputs -> per-core input maps (host-side layout work only)."""
    hs = np.asarray(inputs["hidden_states"], dtype=np.float32)
    am = np.asarray(inputs["attention_mask"], dtype=np.float32)
    Wq = np.asarray(inputs["Wq"], dtype=np.float32)
    Wk = np.asarray(inputs["Wk"], dtype=np.float32)
    Wv = np.asarray(inputs["Wv"], dtype=np.float32)
    bq = np.asarray(inputs["bq"], dtype=np.float32)
    bk = np.asarray(inputs["bk"], dtype=np.float32)
    bv = np.asarray(inputs["bv"], dtype=np.float32)
    Aq = np.asarray(inputs["Aq"], dtype=np.float32)
    Bq = np.asarray(inputs["Bq"], dtype=np.float32)
    Av = np.asarray(inputs["Av"], dtype=np.float32)
    Bv = np.asarray(inputs["Bv"], dtype=np.float32)

    c = np.ascontiguousarray
    xTs = [c(hs[b].T) for b in range(B)]
    in_maps = []
    for core in range(NCORES):
        b, g = core // GPB, core % GPB
        sl = slice(g * DPC, (g + 1) * DPC)
        in_maps.append(
            {
                "xT": xTs[b],
                "wqT": c(Wq[sl].T),
                "wkT": c(Wk[sl].T),
                "wvT": c(Wv[sl].T),
                "aq": c(Aq),
                "av": c(Av),
                "bqT": c(Bq[sl].T),
                "bvT": c(Bv[sl].T),
                "biasq": c(bq[sl]),
                "biask": c(bk[sl]),
                "biasv": c(bv[sl]),
                "amask": c(am[b, 0, 0, :]),
            }
        )
    return in_maps


def _run(inputs, trace=False):
    from concourse.bass_utils import run_bass_kernel_spmd

    if "nc" not in _CACHE:
        _CACHE["nc"] = _build_program()
    nc = _CACHE["nc"]
    in_maps = _shard_inputs(inputs)
    res = run_bass_kernel_spmd(nc, in_maps, list(range(NCORES)), trace=trace)
    out = np.empty((B, T, DM), dtype=np.float32)
    for core in range(NCORES):
        b, g = core // GPB, core % GPB
        out[b, :, g * DPC : (g + 1) * DPC] = res.results[core]["outT"].T
    return out, res


def kernel(**inputs) -> np.ndarray:
    out, _ = _run(inputs, trace=False)
    return out
